# revision 42
# baseline (speedup 1.0000x reference)
"""Trainium2 Bass kernel for a dense transformer block.

Layout strategy: channel-major activations ([d, tokens]) so every linear
layer is a natural PE matmul (contraction dim on partitions, weights in
natural [d_in, d_out] layout as lhsT). Softmax is computed transposed
(S^T = [key, q]) without max-subtraction (scores bounded), with row-sums
obtained from a ones-column appended to V during the A@V matmul.

Sharding over 8 cores, no collectives: core c -> batch b=c//4, query
chunks {j, 7-j} (j=c%4, 256 tokens each). LN1/K/V computed redundantly
for the full batch on each core; causality via per-core mask inputs so
the compiled program is identical on all cores (single-NEFF SPMD).

v1 perf restructure vs baseline:
- LN1 + K/V projections fused into one dense per-slice PE stream;
  V-proj moved out of the attention loop (frees PSUM banks).
- LN stats col-tiled: Sigma-x at PSUM partition 0 and Sigma-x^2 at
  partition 32 run concurrently on the PE array.
- Attention column layout [h0_qA | h1_qA | h0_qB | h1_qB] makes every
  exp/mask op contiguous; for kci<8 only the qA half needs masking
  (qB tokens are at >=1024 and see all keys 0..1023 on every core),
  for kci>=8 only the qB half is computed.
- Score PSUM and A@V accumulators double-buffered (8 banks total),
  removing the inter-pair pipeline stall.
"""

import numpy as np
import ml_dtypes

# Problem constants (hardcoded per task contract)
B, S, D, H, HS, FF = 2, 2048, 1024, 16, 64, 4096
P = 128
ND = D // P          # 8 d-chunks
NT = S // P          # 16 key chunks
NPAIR = H // 2       # 8 head pairs
QW = 256             # query chunk width
OWN = 2 * QW         # 512 owned query tokens per core
NKC0, NKC1 = 8, 16   # key-chunk counts: full window / qB-only window
NFF = FF // P        # 32
EPS = 1e-5
N_CORES = 8

BF16 = ml_dtypes.bfloat16


def build(nc):
    """Build the single-core SPMD program (identical for all cores)."""
    import concourse.mybir as mybir
    from concourse.tile import TileContext
    from contextlib import ExitStack

    dt = mybir.dt
    f32, bf16 = dt.float32, dt.bfloat16
    Exp = mybir.ActivationFunctionType.Exp
    Gelu = mybir.ActivationFunctionType.Gelu
    Sqrt = mybir.ActivationFunctionType.Sqrt
    Identity = mybir.ActivationFunctionType.Identity

    # ---- I/O ----
    xT_d = nc.dram_tensor("xT", [D, S], bf16, kind="ExternalInput")
    xo_d = nc.dram_tensor("xo", [D, OWN], bf16, kind="ExternalInput")
    xof_d = nc.dram_tensor("xof", [D, OWN], f32, kind="ExternalInput")
    wq_d = nc.dram_tensor("wq", [D, D], bf16, kind="ExternalInput")
    wk_d = nc.dram_tensor("wk", [D, D], bf16, kind="ExternalInput")
    wv_d = nc.dram_tensor("wv", [D, D], bf16, kind="ExternalInput")
    wp_d = nc.dram_tensor("wp", [D, D], bf16, kind="ExternalInput")
    w1_d = nc.dram_tensor("w1", [D, FF], bf16, kind="ExternalInput")
    w2_d = nc.dram_tensor("w2", [FF, D], bf16, kind="ExternalInput")
    bq_d = nc.dram_tensor("bq", [P, ND], f32, kind="ExternalInput")
    bk_d = nc.dram_tensor("bk", [P, ND], f32, kind="ExternalInput")
    bv_d = nc.dram_tensor("bv", [P, ND], f32, kind="ExternalInput")
    bp_d = nc.dram_tensor("bp", [P, ND], f32, kind="ExternalInput")
    b1_d = nc.dram_tensor("b1", [P, NFF], f32, kind="ExternalInput")
    b2_d = nc.dram_tensor("b2", [P, ND], f32, kind="ExternalInput")
    # masks[kci]: kci<8 -> [mA|mA] (qA causal mask, dup for 2 heads)
    #            kci>=8 -> [mB|mB]
    mk_d = nc.dram_tensor("masks", [NKC1, P, 2 * QW], bf16,
                          kind="ExternalInput")
    out_d = nc.dram_tensor("outT", [D, OWN], f32, kind="ExternalOutput")

    with TileContext(nc) as tc, ExitStack() as top:
        const = top.enter_context(tc.tile_pool(name="const", bufs=1))
        rowp = top.enter_context(tc.tile_pool(name="rows", bufs=1))

        ones_bf = const.tile([P, 1], bf16)
        nc.vector.memset(ones_bf, 1.0)
        eps_t = const.tile([1, 1], f32)
        nc.vector.memset(eps_t, EPS)

        bias = {}
        for name, dram, w in (("bq", bq_d, ND), ("bk", bk_d, ND),
                              ("bv", bv_d, ND), ("bp", bp_d, ND),
                              ("b1", b1_d, NFF), ("b2", b2_d, ND)):
            t = const.tile([P, w], f32, tag=f"bias_{name}", name=f"bias_{name}")
            nc.sync.dma_start(out=t, in_=dram[:, :])
            bias[name] = t

        def pool_open(**kw):
            cm = tc.tile_pool(**kw)
            return cm, cm.__enter__()

        def pool_close(*cms):
            for cm in cms:
                cm.__exit__(None, None, None)

        def ln_rows(n, sx_ps, sq_ps):
            """row stats [1, n] from Sigma-x (row 0) / Sigma-x2 (row 32 of
            its own bank, col-tiled) -> (r_bf, s_bf)."""
            mean = rowp.tile([1, n], f32, tag="mean", name="mean")
            nc.scalar.mul(mean, sx_ps, 1.0 / D)
            var = rowp.tile([1, n], f32, tag="var", name="var")
            nc.scalar.mul(var, sq_ps[32:33, :], 1.0 / D)
            msq = rowp.tile([1, n], f32, tag="msq", name="msq")
            nc.vector.tensor_mul(msq, mean, mean)
            nc.vector.tensor_sub(var, var, msq)
            std = rowp.tile([1, n], f32, tag="std", name="std")
            nc.scalar.activation(std, var, Sqrt, bias=eps_t)
            r_row = rowp.tile([1, n], f32, tag="r_row", name="r_row")
            nc.vector.reciprocal_approx_fast(r_row, std)
            s_row = rowp.tile([1, n], f32, tag="s_row", name="s_row")
            nc.vector.tensor_mul(s_row, mean, r_row)
            nc.scalar.mul(s_row, s_row, -1.0)
            r_bf = rowp.tile([1, n], bf16, tag="r_bf", name="r_bf")
            nc.vector.tensor_copy(r_bf, r_row)
            s_bf = rowp.tile([1, n], bf16, tag="s_bf", name="s_bf")
            nc.vector.tensor_copy(s_bf, s_row)
            return r_bf, s_bf

        # ---------- long-lived pools ----------
        at_cm, at_pool = pool_open(name="attn", bufs=1)
        attn = [at_pool.tile([P, OWN], bf16, tag=f"at{p}", name=f"at{p}")
                for p in range(NPAIR)]

        # K/V/Q outputs (left), live through attention
        kT_cm, kT_pool = pool_open(name="kT", bufs=1)
        v_cm, v_pool = pool_open(name="v65", bufs=1)
        qT_cm, qT_pool = pool_open(name="qT", bufs=1)
        KT = [kT_pool.tile([P, S], bf16, tag=f"k{p}", name=f"k{p}")
              for p in range(NPAIR)]
        V65 = [v_pool.tile([P, H, HS + 1], bf16, tag=f"v{k}", name=f"v{k}")
               for k in range(NT)]
        QT = [qT_pool.tile([P, OWN], bf16, tag=f"q{p}", name=f"q{p}")
              for p in range(NPAIR)]

        # ===== Fused phase A+C: LN1 -> hT -> K/V proj, per 512-slice =====
        # wk + hT survive into attention: K-proj for pairs 2..7 is injected
        # there to fill PE idle while exp runs on the Scalar engine.
        wk_cm, wk_pool = pool_open(name="wgtK", bufs=1, side="right")
        wk_t = [wk_pool.tile([P, D], bf16, tag=f"wk{i}", name=f"wk{i}")
                for i in range(ND)]
        hT_cm, hT_pool = pool_open(name="hT", bufs=1, side="right")
        hT = [hT_pool.tile([P, S], bf16, tag=f"h{i}", name=f"h{i}")
              for i in range(ND)]

        with tc.tile_pool(name="xin", bufs=2, side="right") as x_pool, \
             tc.tile_pool(name="wgt", bufs=1, side="right") as wpool, \
             tc.tile_pool(name="sq", bufs=3, side="right") as sq_pool, \
             tc.tile_pool(name="htm", bufs=3, side="right") as ht_pool, \
             tc.tile_pool(name="bcA", bufs=2, side="right") as bc_pool, \
             tc.tile_pool(name="psA", bufs=1, space="PSUM") as psA, \
             tc.tile_pool(name="psK", bufs=2, space="PSUM") as psK, \
             tc.tile_pool(name="psV", bufs=2, space="PSUM") as psV:

            # weight loads (wk/wv needed from slice 0; wq after)
            xo_t = [wpool.tile([P, OWN], bf16, tag=f"xo{i}", name=f"xo{i}")
                    for i in range(ND)]
            wv_t = [wpool.tile([P, D], bf16, tag=f"wv{i}", name=f"wv{i}")
                    for i in range(ND)]
            for i in range(ND):
                nc.sync.dma_start(out=wk_t[i], in_=wk_d[P * i:P * i + P, :])
            for i in range(ND):
                nc.sync.dma_start(out=wv_t[i], in_=wv_d[P * i:P * i + P, :])
            for i in range(ND):
                nc.sync.dma_start(out=xo_t[i], in_=xo_d[P * i:P * i + P, :])

            # xt tiles: per-slice, rotating (bufs=2)
            def load_slice(s):
                xt = [x_pool.tile([P, 512], bf16, tag=f"x{i}",
                                  name=f"x{i}s{s}") for i in range(ND)]
                for i in range(ND):
                    nc.sync.dma_start(
                        out=xt[i],
                        in_=xT_d[P * i:P * i + P, 512 * s:512 * s + 512])
                return xt

            xt_next = load_slice(0)

            def emit_stats_chain(s, xt):
                sl = slice(512 * s, 512 * s + 512)
                sx_ps = psA.tile([1, 512], f32, tag="sx")
                sq_ps = psA.tile([33, 512], f32, tag="sq")
                for i in range(ND):
                    sqt = sq_pool.tile([P, 512], bf16, tag="sqt")
                    nc.scalar.square(sqt, xt[i])
                    nc.tensor.matmul(sx_ps, ones_bf, xt[i],
                                     start=(i == 0), stop=(i == ND - 1))
                    nc.tensor.matmul(sq_ps[32:33, :], ones_bf, sqt,
                                     start=(i == 0), stop=(i == ND - 1))
                r_bf, s_bf = ln_rows(512, sx_ps, sq_ps)
                rb = bc_pool.tile([P, 512], bf16, tag="rb")
                nc.gpsimd.partition_broadcast(rb, r_bf)
                sb = bc_pool.tile([P, 512], bf16, tag="sb")
                nc.gpsimd.partition_broadcast(sb, s_bf)
                for i in range(ND):
                    tmp = ht_pool.tile([P, 512], bf16, tag="htmp")
                    nc.vector.tensor_mul(tmp, xt[i], rb)
                    nc.vector.tensor_add(hT[i][:, sl], tmp, sb)

            def emit_kv(s):
                sl = slice(512 * s, 512 * s + 512)
                for pr in range(2):
                    kps = psK.tile([P, 512], f32, tag="kps")
                    for i in range(ND):
                        nc.tensor.matmul(
                            kps, wk_t[i][:, P * pr:P * pr + P],
                            hT[i][:, sl],
                            start=(i == 0), stop=(i == ND - 1))
                    nc.vector.tensor_scalar_add(
                        KT[pr][:, sl], kps, bias["bk"][:, pr:pr + 1])
                for kc in range(4 * s, 4 * s + 4):
                    vps = psV.tile([P, D], f32, tag="vps")
                    for i in range(ND):
                        for nh in range(2):
                            nsl = slice(512 * nh, 512 * nh + 512)
                            nc.tensor.matmul(
                                vps[:, nsl],
                                hT[i][:, P * kc:P * kc + P],
                                wv_t[i][:, nsl],
                                start=(i == 0), stop=(i == ND - 1))
                    nc.vector.memset(V65[kc][:, :, HS:HS + 1], 1.0)
                    nc.vector.tensor_copy(
                        V65[kc][:, :, 0:HS],
                        vps.rearrange("p (h e) -> p h e", e=HS))

            # software-pipelined: stats(s+1) is emitted before K/V(s) so
            # the LN row chain of s+1 hides under the s projections.
            prev = None
            for s in range(S // 512):
                xt = xt_next
                if s + 1 < S // 512:
                    xt_next = load_slice(s + 1)
                emit_stats_chain(s, xt)
                if prev is not None:
                    emit_kv(prev)
                prev = s
            emit_kv(prev)

            # ---- LN-own + Q-proj ----
            wq_t = [wpool.tile([P, D], bf16, tag=f"wv{i}", name=f"wq{i}")
                    for i in range(ND)]
            for i in range(ND):
                nc.sync.dma_start(out=wq_t[i], in_=wq_d[P * i:P * i + P, :])

            with tc.tile_pool(name="hq", bufs=1) as hq_pool:
                hq = [hq_pool.tile([P, OWN], bf16, tag=f"hq{i}",
                                   name=f"hq{i}") for i in range(ND)]
                sx2 = psA.tile([1, OWN], f32, tag="sx")
                sq2 = psA.tile([33, OWN], f32, tag="sq")
                for i in range(ND):
                    sqt = sq_pool.tile([P, OWN], bf16, tag="sqt")
                    nc.scalar.square(sqt, xo_t[i])
                    nc.tensor.matmul(sx2, ones_bf, xo_t[i],
                                     start=(i == 0), stop=(i == ND - 1))
                    nc.tensor.matmul(sq2[32:33, :], ones_bf, sqt,
                                     start=(i == 0), stop=(i == ND - 1))
                r_bf, s_bf = ln_rows(OWN, sx2, sq2)
                rb2 = bc_pool.tile([P, OWN], bf16, tag="rb")
                nc.gpsimd.partition_broadcast(rb2, r_bf)
                sb2 = bc_pool.tile([P, OWN], bf16, tag="sb")
                nc.gpsimd.partition_broadcast(sb2, s_bf)
                for i in range(ND):
                    tmp = ht_pool.tile([P, OWN], bf16, tag="htmp")
                    nc.vector.tensor_mul(tmp, xo_t[i], rb2)
                    nc.vector.tensor_add(hq[i], tmp, sb2)

                for pr in range(NPAIR):
                    qps = psK.tile([P, OWN], f32, tag="kps")
                    for i in range(ND):
                        nc.tensor.matmul(qps,
                                         wq_t[i][:, P * pr:P * pr + P],
                                         hq[i],
                                         start=(i == 0), stop=(i == ND - 1))
                    nc.vector.tensor_scalar_add(QT[pr], qps,
                                                bias["bq"][:, pr:pr + 1])

        # wp weights (phase E) + f32 own-x (residual); DMA'd during
        # attention. hT/wk stay live for the injected K-proj.
        wgtE_cm, wpoolE = pool_open(name="wgtE", bufs=1, side="right")
        wp_t = [wpoolE.tile([P, D], bf16, tag=f"wp{i}", name=f"wp{i}")
                for i in range(ND)]
        xoF_cm, xoF_pool = pool_open(name="xoF", bufs=1, side="right")
        xof_t = [xoF_pool.tile([P, OWN], f32, tag=f"xof{i}", name=f"xof{i}")
                 for i in range(ND)]

        # =========== Attention ===========
        # at columns (baseline layout): [h0: qA qB | h1: qA qB]
        # masks content: kci<8 -> [mA|mA]; kci>=8 -> [mB|mB]; applied with
        # a single 3-level-AP mul per kci (qA blocks or qB blocks only).
        W2Q = 2 * QW
        with tc.tile_pool(name="msk", bufs=1) as mpool, \
             tc.tile_pool(name="atile", bufs=2) as apool, \
             tc.tile_pool(name="rec", bufs=1) as rpool, \
             tc.tile_pool(name="psD", bufs=1, space="PSUM") as psD, \
             tc.tile_pool(name="psKa", bufs=2, space="PSUM") as psKa, \
             tc.tile_pool(name="psS", bufs=2, space="PSUM") as psS:
            mk_t = [mpool.tile([P, 2 * QW], bf16, tag=f"m{u}", name=f"m{u}")
                    for u in range(NKC1)]
            for u in range(NKC1):
                nc.sync.dma_start(out=mk_t[u], in_=mk_d[u])
            # prefetch wp + f32 own-x (residual) during attention
            for i in range(ND):
                nc.sync.dma_start(out=wp_t[i], in_=wp_d[P * i:P * i + P, :])
            for i in range(ND):
                nc.sync.dma_start(out=xof_t[i],
                                  in_=xof_d[P * i:P * i + P, :])

            def emit_scores(pr, kci):
                sps = psS.tile([P, 2 * W2Q], f32, tag="sps", name="sps")
                at = apool.tile([P, 2 * W2Q], bf16, tag="a", name="a")
                kch = slice(P * kci, P * kci + P)
                if kci < NKC0:
                    for h in range(2):
                        hb = slice(64 * h, 64 * h + 64)
                        nc.tensor.matmul(
                            sps[:, W2Q * h:W2Q * h + W2Q],
                            KT[pr][hb, kch], QT[pr][hb, :])
                else:
                    for h in range(2):
                        hb = slice(64 * h, 64 * h + 64)
                        nc.tensor.matmul(
                            sps[:, QW + QW * h:QW + QW * h + QW],
                            KT[pr][hb, kch], QT[pr][hb, QW:OWN])
                return sps, at

            def emit_kslice(pr2, s2):
                # one K-proj output slice for pair pr2, injected into the
                # attention stream to fill PE while ACT/DVE run.
                sl2 = slice(512 * s2, 512 * s2 + 512)
                kps = psKa.tile([P, 512], f32, tag="kps")
                for i in range(ND):
                    nc.tensor.matmul(
                        kps, wk_t[i][:, P * pr2:P * pr2 + P],
                        hT[i][:, sl2],
                        start=(i == 0), stop=(i == ND - 1))
                nc.vector.tensor_scalar_add(
                    KT[pr2][:, sl2], kps, bias["bk"][:, pr2:pr2 + 1])

            steps = [(pr, kci) for pr in range(NPAIR)
                     for kci in range(NKC1)]
            avs = {}
            pending = emit_scores(*steps[0])
            for idx, (pr, kci) in enumerate(steps):
                sps, at = pending
                if idx + 1 < len(steps):
                    pending = emit_scores(*steps[idx + 1])

                if kci == 0:
                    avs[pr] = [psD.tile([HS + 1, OWN], f32, tag=f"av{h}",
                                        name=f"av{h}") for h in range(2)]
                av = avs[pr]
                at4 = at.rearrange("p (h x q) -> p h x q", h=2, x=2)
                mk2 = mk_t[kci].rearrange("p (h q) -> p h q", h=2)
                if kci < NKC0:
                    nc.scalar.activation(at, sps, Exp)
                    # mask qA blocks of both heads in one op
                    nc.vector.tensor_mul(at4[:, :, 0, :],
                                         at4[:, :, 0, :], mk2)
                    for h in range(2):
                        nc.tensor.matmul(
                            av[h], V65[kci][:, 2 * pr + h, :],
                            at[:, W2Q * h:W2Q * h + W2Q],
                            start=(kci == 0), stop=(kci == NKC1 - 1))
                else:
                    nc.scalar.activation(at[:, QW:QW + OWN],
                                         sps[:, QW:QW + OWN], Exp)
                    nc.vector.tensor_mul(at[:, QW:QW + OWN],
                                         at[:, QW:QW + OWN], mk_t[kci])
                    for h in range(2):
                        nc.tensor.matmul(
                            av[h][:, QW:OWN], V65[kci][:, 2 * pr + h, :],
                            at[:, QW + QW * h:QW + QW * h + QW],
                            start=False, stop=(kci == NKC1 - 1))
                if kci == NKC1 - 1:
                    # normalize: copy sums to SBUF, reciprocal, broadcast
                    sums = rpool.tile([1, 2 * W2Q], f32, tag="sums")
                    nc.vector.tensor_copy(sums[:, 0:OWN],
                                          av[0][HS:HS + 1, :])
                    nc.vector.tensor_copy(sums[:, OWN:2 * OWN],
                                          av[1][HS:HS + 1, :])
                    rec = rpool.tile([1, 2 * W2Q], f32, tag="rec")
                    nc.vector.reciprocal_approx_fast(rec, sums)
                    rb_sb = rpool.tile([64, 2 * W2Q], f32, tag="rb_sb")
                    nc.gpsimd.partition_broadcast(rb_sb, rec)
                    for h in range(2):
                        hb = slice(64 * h, 64 * h + 64)
                        nc.vector.tensor_mul(
                            attn[pr][hb, :], av[h][0:HS, :],
                            rb_sb[:, OWN * h:OWN * h + OWN])
                        nc.vector.tensor_scalar_add(
                            attn[pr][hb, :], attn[pr][hb, :],
                            bias["bv"][64 * h:64 * h + 64, pr:pr + 1])
                    del avs[pr]
                if kci >= NKC1 - 4 and pr + 2 < NPAIR:
                    emit_kslice(pr + 2, kci - (NKC1 - 4))

        pool_close(qT_cm, v_cm, kT_cm)

        # right-stack pools for the token-parallel tail
        x2_cm, x2_pool = pool_open(name="x2", bufs=1, side="right")
        x2f = [x2_pool.tile([P, OWN], f32, tag=f"x2f{i}", name=f"x2f{i}")
               for i in range(ND)]
        x2b = [x2_pool.tile([P, OWN], bf16, tag=f"x2b{i}", name=f"x2b{i}")
               for i in range(ND)]

        # ====== Phase E+F fused: proj + residual + LN2 (pipelined) ======
        h2_cm, h2_pool = pool_open(name="h2", bufs=1, side="right")
        h2 = [h2_pool.tile([P, OWN], bf16, tag=f"h2{i}", name=f"h2{i}")
              for i in range(ND)]
        with tc.tile_pool(name="sqF", bufs=3, side="right") as sqF, \
             tc.tile_pool(name="htmF", bufs=3, side="right") as htF, \
             tc.tile_pool(name="bcF", bufs=1, side="right") as bcF, \
             tc.tile_pool(name="psE", bufs=2, space="PSUM") as psE, \
             tc.tile_pool(name="psF", bufs=1, space="PSUM") as psF:
            sxF = psF.tile([1, OWN], f32, tag="sxF")
            sqFp = psF.tile([33, OWN], f32, tag="sqFp")

            def emit_statF(mc):
                sqt = sqF.tile([P, OWN], bf16, tag="sqtF")
                nc.scalar.square(sqt, x2b[mc])
                nc.tensor.matmul(sxF, ones_bf, x2b[mc],
                                 start=(mc == 0), stop=(mc == ND - 1))
                nc.tensor.matmul(sqFp[32:33, :], ones_bf, sqt,
                                 start=(mc == 0), stop=(mc == ND - 1))

            for mc in range(ND):
                ops = psE.tile([P, OWN], f32, tag="ops")
                for i in range(NPAIR):
                    nc.tensor.matmul(ops, wp_t[i][:, P * mc:P * mc + P],
                                     attn[i],
                                     start=(i == 0), stop=(i == NPAIR - 1))
                nc.scalar.activation(x2f[mc], ops, Identity,
                                     bias=bias["bp"][:, mc:mc + 1])
                nc.vector.tensor_add(x2f[mc], x2f[mc], xof_t[mc])
                nc.vector.tensor_copy(x2b[mc], x2f[mc])
                if mc >= 1:
                    emit_statF(mc - 1)
            emit_statF(ND - 1)
            r_bf, s_bf = ln_rows(OWN, sxF, sqFp)
            rbF = bcF.tile([P, OWN], bf16, tag="rbF")
            nc.gpsimd.partition_broadcast(rbF, r_bf)
            sbF = bcF.tile([P, OWN], bf16, tag="sbF")
            nc.gpsimd.partition_broadcast(sbF, s_bf)
            for i in range(ND):
                tmp = htF.tile([P, OWN], bf16, tag="htmpF")
                nc.vector.tensor_mul(tmp, x2b[i], rbF)
                nc.vector.tensor_add(h2[i], tmp, sbF)

        pool_close(at_cm)

        # =========== Phase G: FFN ===========
        with tc.tile_pool(name="gbuf", bufs=1, side="right") as g_pool, \
             tc.tile_pool(name="w1c", bufs=4, side="right") as w1pool, \
             tc.tile_pool(name="w2c", bufs=3, side="right") as w2pool, \
             tc.tile_pool(name="outp", bufs=4, side="right") as opool, \
             tc.tile_pool(name="psG", bufs=2, space="PSUM") as psG:
            g = [g_pool.tile([P, OWN], bf16, tag=f"g{m}", name=f"g{m}")
                 for m in range(NFF)]
            w1r = w1_d.rearrange("(ko ki) f -> ki ko f", ki=P)
            for mc in range(NFF):
                w1c = w1pool.tile([P, ND, P], bf16, tag="w1c")
                nc.sync.dma_start(out=w1c, in_=w1r[:, :, P * mc:P * mc + P])
                ups = psG.tile([P, OWN], f32, tag="ups")
                for i in range(ND):
                    nc.tensor.matmul(ups, w1c[:, i, :], h2[i],
                                     start=(i == 0), stop=(i == ND - 1))
                nc.scalar.activation(g[mc], ups, Gelu,
                                     bias=bias["b1"][:, mc:mc + 1])
            w2r = w2_d.rearrange("(ko ki) f -> ki ko f", ki=P)
            for oc in range(ND):
                w2c = w2pool.tile([P, NFF, P], bf16, tag="w2c")
                nc.sync.dma_start(out=w2c, in_=w2r[:, :, P * oc:P * oc + P])
                wps = psG.tile([P, OWN], f32, tag="ups")
                for k in range(NFF):
                    nc.tensor.matmul(wps, w2c[:, k, :], g[k],
                                     start=(k == 0), stop=(k == NFF - 1))
                of = opool.tile([P, OWN], f32, tag="of")
                nc.scalar.activation(of, wps, Identity,
                                     bias=bias["b2"][:, oc:oc + 1])
                nc.vector.tensor_add(of, of, x2f[oc])
                nc.sync.dma_start(out=out_d[P * oc:P * oc + P, :], in_=of)

        pool_close(h2_cm, x2_cm, xoF_cm, wgtE_cm, hT_cm, wk_cm)

    return nc


def host_prep(inputs):
    """Build per-core input maps + gather metadata. Pure numpy."""
    x = np.asarray(inputs["x"], np.float32)
    ln1_w = np.asarray(inputs["ln1_w"], np.float32)
    ln1_b = np.asarray(inputs["ln1_b"], np.float32)
    ln2_w = np.asarray(inputs["ln2_w"], np.float32)
    ln2_b = np.asarray(inputs["ln2_b"], np.float32)

    def cat_heads(w):
        return np.ascontiguousarray(
            np.transpose(np.asarray(w, np.float32), (1, 0, 2)).reshape(D, D))

    wq_c, wk_c, wv_c = (cat_heads(inputs[k]) for k in ("Wq", "Wk", "Wv"))
    bq_f = np.asarray(inputs["bq"], np.float32).reshape(-1)
    bk_f = np.asarray(inputs["bk"], np.float32).reshape(-1)
    bv_f = np.asarray(inputs["bv"], np.float32).reshape(-1)
    Wp = np.asarray(inputs["Wp"], np.float32)
    bp = np.asarray(inputs["bp"], np.float32)
    W1 = np.asarray(inputs["W1"], np.float32)
    b1 = np.asarray(inputs["b1"], np.float32)
    W2 = np.asarray(inputs["W2"], np.float32)
    b2 = np.asarray(inputs["b2"], np.float32)

    sc = 1.0 / np.sqrt(HS)
    wq_eff = ((ln1_w[:, None] * wq_c) * sc).astype(BF16)
    bq_eff = ((ln1_b @ wq_c + bq_f) * sc).astype(np.float32)
    wk_eff = (ln1_w[:, None] * wk_c).astype(BF16)
    bk_eff = (ln1_b @ wk_c + bk_f).astype(np.float32)
    wv_eff = (ln1_w[:, None] * wv_c).astype(BF16)
    bv_eff = (ln1_b @ wv_c + bv_f).astype(np.float32)
    wp_eff = Wp.astype(BF16)
    w1_eff = (ln2_w[:, None] * W1).astype(BF16)
    b1_eff = (ln2_b @ W1 + b1).astype(np.float32)
    w2_eff = W2.astype(BF16)

    def chunked(v, n):
        return np.ascontiguousarray(v.reshape(n, P).T).astype(np.float32)

    shared = {
        "wq": wq_eff, "wk": wk_eff, "wv": wv_eff, "wp": wp_eff,
        "w1": w1_eff, "w2": w2_eff,
        "bq": chunked(bq_eff, ND), "bk": chunked(bk_eff, ND),
        "bv": chunked(bv_eff, ND), "bp": chunked(bp, ND),
        "b1": chunked(b1_eff, NFF), "b2": chunked(b2, ND),
    }

    in_maps, gathers = [], []
    for c in range(N_CORES):
        b, j = c // 4, c % 4
        qA, qB = QW * j, QW * (7 - j)
        xT = np.ascontiguousarray(x[b].T).astype(BF16)
        xo_f = np.ascontiguousarray(
            np.concatenate([x[b, qA:qA + QW].T, x[b, qB:qB + QW].T],
                           axis=1)).astype(np.float32)
        # masks: kci<8 -> qA causal mask duplicated for both heads;
        #        kci>=8 -> qB causal mask duplicated.
        ks = np.arange(P)[:, None]
        qs = np.arange(QW)[None, :]
        masks = np.zeros((NKC1, P, 2 * QW), np.float32)
        for kc in range(NKC1):
            if kc < NKC0:
                m = (P * kc + ks) <= (qA + qs)
            else:
                m = (P * kc + ks) <= (qB + qs)
            masks[kc, :, 0:QW] = m
            masks[kc, :, QW:2 * QW] = m
        m = dict(shared)
        m["xT"] = xT
        m["xo"] = xo_f.astype(BF16)
        m["xof"] = xo_f
        m["masks"] = masks.astype(BF16)
        in_maps.append(m)
        gathers.append((b, qA, qB))
    return in_maps, gathers


def make_nc():
    from concourse import bacc

    nc = bacc.Bacc("TRN2")
    build(nc)
    nc.compile()
    return nc


def kernel(**inputs):
    from concourse.bass_utils import run_bass_kernel_spmd

    nc = make_nc()
    in_maps, gathers = host_prep(inputs)
    res = run_bass_kernel_spmd(nc, in_maps, list(range(N_CORES)))
    out = np.zeros((B, S, D), np.float32)
    for c, (b, qA, qB) in enumerate(gathers):
        oT = res.results[c]["outT"]
        out[b, qA:qA + QW] = oT[:, 0:QW].T
        out[b, qB:qB + QW] = oT[:, QW:2 * QW].T
    return out


# revision 43
# speedup vs baseline: 1.0123x; 1.0123x over previous
"""Trainium2 Bass kernel for a dense transformer block.

Layout strategy: channel-major activations ([d, tokens]) so every linear
layer is a natural PE matmul (contraction dim on partitions, weights in
natural [d_in, d_out] layout as lhsT). Softmax is computed transposed
(S^T = [key, q]) without max-subtraction (scores bounded), with row-sums
obtained from a ones-column appended to V during the A@V matmul.

Sharding over 8 cores, no collectives: core c -> batch b=c//4, query
chunks {j, 7-j} (j=c%4, 256 tokens each). LN1/K/V computed redundantly
for the full batch on each core; causality via per-core mask inputs so
the compiled program is identical on all cores (single-NEFF SPMD).

v1 perf restructure vs baseline:
- LN1 + K/V projections fused into one dense per-slice PE stream;
  V-proj moved out of the attention loop (frees PSUM banks).
- LN stats col-tiled: Sigma-x at PSUM partition 0 and Sigma-x^2 at
  partition 32 run concurrently on the PE array.
- Attention column layout [h0_qA | h1_qA | h0_qB | h1_qB] makes every
  exp/mask op contiguous; for kci<8 only the qA half needs masking
  (qB tokens are at >=1024 and see all keys 0..1023 on every core),
  for kci>=8 only the qB half is computed.
- Score PSUM and A@V accumulators double-buffered (8 banks total),
  removing the inter-pair pipeline stall.
"""

import numpy as np
import ml_dtypes

# Problem constants (hardcoded per task contract)
B, S, D, H, HS, FF = 2, 2048, 1024, 16, 64, 4096
P = 128
ND = D // P          # 8 d-chunks
NT = S // P          # 16 key chunks
NPAIR = H // 2       # 8 head pairs
QW = 256             # query chunk width
OWN = 2 * QW         # 512 owned query tokens per core
NKC0, NKC1 = 8, 16   # key-chunk counts: full window / qB-only window
NFF = FF // P        # 32
EPS = 1e-5
N_CORES = 8

BF16 = ml_dtypes.bfloat16


def build(nc):
    """Build the single-core SPMD program (identical for all cores)."""
    import concourse.mybir as mybir
    from concourse.tile import TileContext
    from contextlib import ExitStack

    dt = mybir.dt
    f32, bf16 = dt.float32, dt.bfloat16
    Exp = mybir.ActivationFunctionType.Exp
    Gelu = mybir.ActivationFunctionType.Gelu
    Sqrt = mybir.ActivationFunctionType.Sqrt
    Identity = mybir.ActivationFunctionType.Identity

    # ---- I/O ----
    xT_d = nc.dram_tensor("xT", [D, S], bf16, kind="ExternalInput")
    xo_d = nc.dram_tensor("xo", [D, OWN], bf16, kind="ExternalInput")
    xof_d = nc.dram_tensor("xof", [D, OWN], f32, kind="ExternalInput")
    wq_d = nc.dram_tensor("wq", [D, D], bf16, kind="ExternalInput")
    wk_d = nc.dram_tensor("wk", [D, D], bf16, kind="ExternalInput")
    wv_d = nc.dram_tensor("wv", [D, D], bf16, kind="ExternalInput")
    wp_d = nc.dram_tensor("wp", [D, D], bf16, kind="ExternalInput")
    w1_d = nc.dram_tensor("w1", [D, FF], bf16, kind="ExternalInput")
    w2_d = nc.dram_tensor("w2", [FF, D], bf16, kind="ExternalInput")
    bq_d = nc.dram_tensor("bq", [P, ND], f32, kind="ExternalInput")
    bk_d = nc.dram_tensor("bk", [P, ND], f32, kind="ExternalInput")
    bv_d = nc.dram_tensor("bv", [P, ND], f32, kind="ExternalInput")
    bp_d = nc.dram_tensor("bp", [P, ND], f32, kind="ExternalInput")
    b1_d = nc.dram_tensor("b1", [P, NFF], f32, kind="ExternalInput")
    b2_d = nc.dram_tensor("b2", [P, ND], f32, kind="ExternalInput")
    # masks[kci]: kci<8 -> [mA|mA] (qA causal mask, dup for 2 heads)
    #            kci>=8 -> [mB|mB]
    mk_d = nc.dram_tensor("masks", [NKC1, P, 2 * QW], bf16,
                          kind="ExternalInput")
    out_d = nc.dram_tensor("outT", [D, OWN], f32, kind="ExternalOutput")

    with TileContext(nc) as tc, ExitStack() as top:
        const = top.enter_context(tc.tile_pool(name="const", bufs=1))
        rowp = top.enter_context(tc.tile_pool(name="rows", bufs=1))

        ones_bf = const.tile([P, 1], bf16)
        nc.vector.memset(ones_bf, 1.0)
        eps_t = const.tile([1, 1], f32)
        nc.vector.memset(eps_t, EPS)

        bias = {}
        for name, dram, w in (("bq", bq_d, ND), ("bk", bk_d, ND),
                              ("bv", bv_d, ND), ("bp", bp_d, ND),
                              ("b1", b1_d, NFF), ("b2", b2_d, ND)):
            t = const.tile([P, w], f32, tag=f"bias_{name}", name=f"bias_{name}")
            nc.sync.dma_start(out=t, in_=dram[:, :])
            bias[name] = t

        def pool_open(**kw):
            cm = tc.tile_pool(**kw)
            return cm, cm.__enter__()

        def pool_close(*cms):
            for cm in cms:
                cm.__exit__(None, None, None)

        def ln_rows(n, sx_ps, sq_ps):
            """row stats [1, n] from Sigma-x / Sigma-x2 PSUM -> (r_bf, s_bf).
            Tags shared across phases (sequential use)."""
            mean = rowp.tile([1, n], f32, tag="mean", name="mean")
            nc.scalar.mul(mean, sx_ps, 1.0 / D)
            var = rowp.tile([1, n], f32, tag="var", name="var")
            nc.scalar.mul(var, sq_ps, 1.0 / D)
            msq = rowp.tile([1, n], f32, tag="msq", name="msq")
            nc.vector.tensor_mul(msq, mean, mean)
            nc.vector.tensor_sub(var, var, msq)
            std = rowp.tile([1, n], f32, tag="std", name="std")
            nc.scalar.activation(std, var, Sqrt, bias=eps_t)
            r_row = rowp.tile([1, n], f32, tag="r_row", name="r_row")
            nc.vector.reciprocal_approx_fast(r_row, std)
            s_row = rowp.tile([1, n], f32, tag="s_row", name="s_row")
            nc.vector.tensor_mul(s_row, mean, r_row)
            nc.scalar.mul(s_row, s_row, -1.0)
            r_bf = rowp.tile([1, n], bf16, tag="r_bf", name="r_bf")
            nc.vector.tensor_copy(r_bf, r_row)
            s_bf = rowp.tile([1, n], bf16, tag="s_bf", name="s_bf")
            nc.vector.tensor_copy(s_bf, s_row)
            return r_bf, s_bf

        # ---------- long-lived pools ----------
        at_cm, at_pool = pool_open(name="attn", bufs=1)
        attn = [at_pool.tile([P, OWN], bf16, tag=f"at{p}", name=f"at{p}")
                for p in range(NPAIR)]

        # K/V/Q outputs (left), live through attention
        kT_cm, kT_pool = pool_open(name="kT", bufs=1)
        v_cm, v_pool = pool_open(name="v65", bufs=1)
        qT_cm, qT_pool = pool_open(name="qT", bufs=1)
        KT = [kT_pool.tile([P, S], bf16, tag=f"k{p}", name=f"k{p}")
              for p in range(NPAIR)]
        V65 = [v_pool.tile([P, H, HS + 1], bf16, tag=f"v{k}", name=f"v{k}")
               for k in range(NT)]
        QT = [qT_pool.tile([P, OWN], bf16, tag=f"q{p}", name=f"q{p}")
              for p in range(NPAIR)]

        # ===== Fused phase A+C: LN1 -> hT -> K/V proj, per 512-slice =====
        # wk + hT survive into attention: K-proj for pairs 2..7 is injected
        # there to fill PE idle while exp runs on the Scalar engine.
        wk_cm, wk_pool = pool_open(name="wgtK", bufs=1, side="right")
        wk_t = [wk_pool.tile([P, D], bf16, tag=f"wk{i}", name=f"wk{i}")
                for i in range(ND)]
        hT_cm, hT_pool = pool_open(name="hT", bufs=1, side="right")
        hT = [hT_pool.tile([P, S], bf16, tag=f"h{i}", name=f"h{i}")
              for i in range(ND)]

        with tc.tile_pool(name="xin", bufs=2, side="right") as x_pool, \
             tc.tile_pool(name="wgt", bufs=1, side="right") as wpool, \
             tc.tile_pool(name="sq", bufs=3, side="right") as sq_pool, \
             tc.tile_pool(name="htm", bufs=3, side="right") as ht_pool, \
             tc.tile_pool(name="bcA", bufs=2, side="right") as bc_pool, \
             tc.tile_pool(name="psA", bufs=1, space="PSUM") as psA, \
             tc.tile_pool(name="psK", bufs=2, space="PSUM") as psK, \
             tc.tile_pool(name="psV", bufs=2, space="PSUM") as psV:

            # weight loads (wk/wv needed from slice 0; wq after)
            xo_t = [wpool.tile([P, OWN], bf16, tag=f"xo{i}", name=f"xo{i}")
                    for i in range(ND)]
            wv_t = [wpool.tile([P, D], bf16, tag=f"wv{i}", name=f"wv{i}")
                    for i in range(ND)]
            for i in range(ND):
                nc.sync.dma_start(out=wk_t[i], in_=wk_d[P * i:P * i + P, :])
            for i in range(ND):
                nc.sync.dma_start(out=wv_t[i], in_=wv_d[P * i:P * i + P, :])
            for i in range(ND):
                nc.sync.dma_start(out=xo_t[i], in_=xo_d[P * i:P * i + P, :])

            # xt tiles: per-slice, rotating (bufs=2)
            def load_slice(s):
                xt = [x_pool.tile([P, 512], bf16, tag=f"x{i}",
                                  name=f"x{i}s{s}") for i in range(ND)]
                for i in range(ND):
                    nc.sync.dma_start(
                        out=xt[i],
                        in_=xT_d[P * i:P * i + P, 512 * s:512 * s + 512])
                return xt

            xt_next = load_slice(0)

            def emit_stats_chain(s, xt):
                sl = slice(512 * s, 512 * s + 512)
                sx_ps = psA.tile([1, 512], f32, tag="sx")
                sq_ps = psA.tile([1, 512], f32, tag="sq")
                for i in range(ND):
                    sqt = sq_pool.tile([P, 512], bf16, tag="sqt")
                    nc.scalar.square(sqt, xt[i])
                    nc.tensor.matmul(sx_ps, ones_bf, xt[i],
                                     start=(i == 0), stop=(i == ND - 1))
                    nc.tensor.matmul(sq_ps, ones_bf, sqt,
                                     start=(i == 0), stop=(i == ND - 1))
                r_bf, s_bf = ln_rows(512, sx_ps, sq_ps)
                rb = bc_pool.tile([P, 512], bf16, tag="rb")
                nc.gpsimd.partition_broadcast(rb, r_bf)
                sb = bc_pool.tile([P, 512], bf16, tag="sb")
                nc.gpsimd.partition_broadcast(sb, s_bf)
                for i in range(ND):
                    tmp = ht_pool.tile([P, 512], bf16, tag="htmp")
                    nc.vector.tensor_mul(tmp, xt[i], rb)
                    nc.vector.tensor_add(hT[i][:, sl], tmp, sb)

            def emit_kv(s):
                sl = slice(512 * s, 512 * s + 512)
                for pr in range(2):
                    kps = psK.tile([P, 512], f32, tag="kps")
                    for i in range(ND):
                        nc.tensor.matmul(
                            kps, wk_t[i][:, P * pr:P * pr + P],
                            hT[i][:, sl],
                            start=(i == 0), stop=(i == ND - 1))
                    nc.vector.tensor_scalar_add(
                        KT[pr][:, sl], kps, bias["bk"][:, pr:pr + 1])
                for kc in range(4 * s, 4 * s + 4):
                    vps = psV.tile([P, D], f32, tag="vps")
                    for i in range(ND):
                        for nh in range(2):
                            nsl = slice(512 * nh, 512 * nh + 512)
                            nc.tensor.matmul(
                                vps[:, nsl],
                                hT[i][:, P * kc:P * kc + P],
                                wv_t[i][:, nsl],
                                start=(i == 0), stop=(i == ND - 1))
                    nc.vector.memset(V65[kc][:, :, HS:HS + 1], 1.0)
                    nc.vector.tensor_copy(
                        V65[kc][:, :, 0:HS],
                        vps.rearrange("p (h e) -> p h e", e=HS))

            # software-pipelined: stats(s+1) is emitted before K/V(s) so
            # the LN row chain of s+1 hides under the s projections.
            prev = None
            for s in range(S // 512):
                xt = xt_next
                if s + 1 < S // 512:
                    xt_next = load_slice(s + 1)
                emit_stats_chain(s, xt)
                if prev is not None:
                    emit_kv(prev)
                prev = s
            emit_kv(prev)

            # ---- LN-own + Q-proj ----
            wq_t = [wpool.tile([P, D], bf16, tag=f"wv{i}", name=f"wq{i}")
                    for i in range(ND)]
            for i in range(ND):
                nc.sync.dma_start(out=wq_t[i], in_=wq_d[P * i:P * i + P, :])

            with tc.tile_pool(name="hq", bufs=1) as hq_pool:
                hq = [hq_pool.tile([P, OWN], bf16, tag=f"hq{i}",
                                   name=f"hq{i}") for i in range(ND)]
                sx2 = psA.tile([1, OWN], f32, tag="sx")
                sq2 = psA.tile([1, OWN], f32, tag="sq")
                for i in range(ND):
                    sqt = sq_pool.tile([P, OWN], bf16, tag="sqt")
                    nc.scalar.square(sqt, xo_t[i])
                    nc.tensor.matmul(sx2, ones_bf, xo_t[i],
                                     start=(i == 0), stop=(i == ND - 1))
                    nc.tensor.matmul(sq2, ones_bf, sqt,
                                     start=(i == 0), stop=(i == ND - 1))
                r_bf, s_bf = ln_rows(OWN, sx2, sq2)
                rb2 = bc_pool.tile([P, OWN], bf16, tag="rb")
                nc.gpsimd.partition_broadcast(rb2, r_bf)
                sb2 = bc_pool.tile([P, OWN], bf16, tag="sb")
                nc.gpsimd.partition_broadcast(sb2, s_bf)
                for i in range(ND):
                    tmp = ht_pool.tile([P, OWN], bf16, tag="htmp")
                    nc.vector.tensor_mul(tmp, xo_t[i], rb2)
                    nc.vector.tensor_add(hq[i], tmp, sb2)

                for pr in range(NPAIR):
                    qps = psK.tile([P, OWN], f32, tag="kps")
                    for i in range(ND):
                        nc.tensor.matmul(qps,
                                         wq_t[i][:, P * pr:P * pr + P],
                                         hq[i],
                                         start=(i == 0), stop=(i == ND - 1))
                    nc.vector.tensor_scalar_add(QT[pr], qps,
                                                bias["bq"][:, pr:pr + 1])

        # wp weights (phase E) + f32 own-x (residual); DMA'd during
        # attention. hT/wk stay live for the injected K-proj.
        wgtE_cm, wpoolE = pool_open(name="wgtE", bufs=1, side="right")
        wp_t = [wpoolE.tile([P, D], bf16, tag=f"wp{i}", name=f"wp{i}")
                for i in range(ND)]
        xoF_cm, xoF_pool = pool_open(name="xoF", bufs=1, side="right")
        xof_t = [xoF_pool.tile([P, OWN], f32, tag=f"xof{i}", name=f"xof{i}")
                 for i in range(ND)]

        # =========== Attention ===========
        # at columns (baseline layout): [h0: qA qB | h1: qA qB]
        # masks content: kci<8 -> [mA|mA]; kci>=8 -> [mB|mB]; applied with
        # a single 3-level-AP mul per kci (qA blocks or qB blocks only).
        W2Q = 2 * QW
        with tc.tile_pool(name="msk", bufs=1) as mpool, \
             tc.tile_pool(name="atile", bufs=2) as apool, \
             tc.tile_pool(name="rec", bufs=1) as rpool, \
             tc.tile_pool(name="psD", bufs=1, space="PSUM") as psD, \
             tc.tile_pool(name="psKa", bufs=2, space="PSUM") as psKa, \
             tc.tile_pool(name="psS", bufs=2, space="PSUM") as psS:
            mk_t = [mpool.tile([P, 2 * QW], bf16, tag=f"m{u}", name=f"m{u}")
                    for u in range(NKC1)]
            for u in range(NKC1):
                nc.sync.dma_start(out=mk_t[u], in_=mk_d[u])
            # prefetch wp + f32 own-x (residual) during attention
            for i in range(ND):
                nc.sync.dma_start(out=wp_t[i], in_=wp_d[P * i:P * i + P, :])
            for i in range(ND):
                nc.sync.dma_start(out=xof_t[i],
                                  in_=xof_d[P * i:P * i + P, :])

            def emit_scores(pr, kci):
                sps = psS.tile([P, 2 * W2Q], f32, tag="sps", name="sps")
                at = apool.tile([P, 2 * W2Q], bf16, tag="a", name="a")
                kch = slice(P * kci, P * kci + P)
                if kci < NKC0:
                    for h in range(2):
                        hb = slice(64 * h, 64 * h + 64)
                        nc.tensor.matmul(
                            sps[:, W2Q * h:W2Q * h + W2Q],
                            KT[pr][hb, kch], QT[pr][hb, :])
                else:
                    for h in range(2):
                        hb = slice(64 * h, 64 * h + 64)
                        nc.tensor.matmul(
                            sps[:, QW + QW * h:QW + QW * h + QW],
                            KT[pr][hb, kch], QT[pr][hb, QW:OWN])
                return sps, at

            def emit_kslice(pr2, s2):
                # one K-proj output slice for pair pr2, injected into the
                # attention stream to fill PE while ACT/DVE run.
                sl2 = slice(512 * s2, 512 * s2 + 512)
                kps = psKa.tile([P, 512], f32, tag="kps")
                for i in range(ND):
                    nc.tensor.matmul(
                        kps, wk_t[i][:, P * pr2:P * pr2 + P],
                        hT[i][:, sl2],
                        start=(i == 0), stop=(i == ND - 1))
                nc.vector.tensor_scalar_add(
                    KT[pr2][:, sl2], kps, bias["bk"][:, pr2:pr2 + 1])

            steps = [(pr, kci) for pr in range(NPAIR)
                     for kci in range(NKC1)]
            avs = {}
            pending = emit_scores(*steps[0])
            for idx, (pr, kci) in enumerate(steps):
                sps, at = pending
                if idx + 1 < len(steps):
                    pending = emit_scores(*steps[idx + 1])

                if kci == 0:
                    avs[pr] = [psD.tile([HS + 1, OWN], f32, tag=f"av{h}",
                                        name=f"av{h}") for h in range(2)]
                av = avs[pr]
                at4 = at.rearrange("p (h x q) -> p h x q", h=2, x=2)
                mk2 = mk_t[kci].rearrange("p (h q) -> p h q", h=2)
                if kci < NKC0:
                    nc.scalar.activation(at, sps, Exp)
                    # mask qA blocks of both heads in one op
                    nc.vector.tensor_mul(at4[:, :, 0, :],
                                         at4[:, :, 0, :], mk2)
                    for h in range(2):
                        nc.tensor.matmul(
                            av[h], V65[kci][:, 2 * pr + h, :],
                            at[:, W2Q * h:W2Q * h + W2Q],
                            start=(kci == 0), stop=(kci == NKC1 - 1))
                else:
                    nc.scalar.activation(at[:, QW:QW + OWN],
                                         sps[:, QW:QW + OWN], Exp)
                    nc.vector.tensor_mul(at[:, QW:QW + OWN],
                                         at[:, QW:QW + OWN], mk_t[kci])
                    for h in range(2):
                        nc.tensor.matmul(
                            av[h][:, QW:OWN], V65[kci][:, 2 * pr + h, :],
                            at[:, QW + QW * h:QW + QW * h + QW],
                            start=False, stop=(kci == NKC1 - 1))
                if kci == NKC1 - 1:
                    # normalize: copy sums to SBUF, reciprocal, broadcast
                    sums = rpool.tile([1, 2 * W2Q], f32, tag="sums")
                    nc.vector.tensor_copy(sums[:, 0:OWN],
                                          av[0][HS:HS + 1, :])
                    nc.vector.tensor_copy(sums[:, OWN:2 * OWN],
                                          av[1][HS:HS + 1, :])
                    rec = rpool.tile([1, 2 * W2Q], f32, tag="rec")
                    nc.vector.reciprocal_approx_fast(rec, sums)
                    rb_sb = rpool.tile([64, 2 * W2Q], f32, tag="rb_sb")
                    nc.gpsimd.partition_broadcast(rb_sb, rec)
                    for h in range(2):
                        hb = slice(64 * h, 64 * h + 64)
                        nc.vector.tensor_mul(
                            attn[pr][hb, :], av[h][0:HS, :],
                            rb_sb[:, OWN * h:OWN * h + OWN])
                        nc.vector.tensor_scalar_add(
                            attn[pr][hb, :], attn[pr][hb, :],
                            bias["bv"][64 * h:64 * h + 64, pr:pr + 1])
                    del avs[pr]
                if kci >= NKC1 - 4 and pr + 2 < NPAIR:
                    emit_kslice(pr + 2, kci - (NKC1 - 4))

        pool_close(qT_cm, v_cm, kT_cm)

        # right-stack pools for the token-parallel tail
        x2_cm, x2_pool = pool_open(name="x2", bufs=1, side="right")
        x2f = [x2_pool.tile([P, OWN], f32, tag=f"x2f{i}", name=f"x2f{i}")
               for i in range(ND)]
        x2b = [x2_pool.tile([P, OWN], bf16, tag=f"x2b{i}", name=f"x2b{i}")
               for i in range(ND)]

        # ====== Phase E+F fused: proj + residual + LN2 (pipelined) ======
        h2_cm, h2_pool = pool_open(name="h2", bufs=1, side="right")
        h2 = [h2_pool.tile([P, OWN], bf16, tag=f"h2{i}", name=f"h2{i}")
              for i in range(ND)]
        with tc.tile_pool(name="sqF", bufs=3, side="right") as sqF, \
             tc.tile_pool(name="htmF", bufs=3, side="right") as htF, \
             tc.tile_pool(name="bcF", bufs=1, side="right") as bcF, \
             tc.tile_pool(name="psE", bufs=2, space="PSUM") as psE, \
             tc.tile_pool(name="psF", bufs=1, space="PSUM") as psF:
            sxF = psF.tile([1, OWN], f32, tag="sxF")
            sqFp = psF.tile([1, OWN], f32, tag="sqFp")

            def emit_statF(mc):
                sqt = sqF.tile([P, OWN], bf16, tag="sqtF")
                nc.scalar.square(sqt, x2b[mc])
                nc.tensor.matmul(sxF, ones_bf, x2b[mc],
                                 start=(mc == 0), stop=(mc == ND - 1))
                nc.tensor.matmul(sqFp, ones_bf, sqt,
                                 start=(mc == 0), stop=(mc == ND - 1))

            for mc in range(ND):
                ops = psE.tile([P, OWN], f32, tag="ops")
                for i in range(NPAIR):
                    nc.tensor.matmul(ops, wp_t[i][:, P * mc:P * mc + P],
                                     attn[i],
                                     start=(i == 0), stop=(i == NPAIR - 1))
                nc.scalar.activation(x2f[mc], ops, Identity,
                                     bias=bias["bp"][:, mc:mc + 1])
                nc.vector.tensor_add(x2f[mc], x2f[mc], xof_t[mc])
                nc.vector.tensor_copy(x2b[mc], x2f[mc])
                if mc >= 1:
                    emit_statF(mc - 1)
            emit_statF(ND - 1)
            r_bf, s_bf = ln_rows(OWN, sxF, sqFp)
            rbF = bcF.tile([P, OWN], bf16, tag="rbF")
            nc.gpsimd.partition_broadcast(rbF, r_bf)
            sbF = bcF.tile([P, OWN], bf16, tag="sbF")
            nc.gpsimd.partition_broadcast(sbF, s_bf)
            for i in range(ND):
                tmp = htF.tile([P, OWN], bf16, tag="htmpF")
                nc.vector.tensor_mul(tmp, x2b[i], rbF)
                nc.vector.tensor_add(h2[i], tmp, sbF)

        pool_close(at_cm)

        # =========== Phase G: FFN ===========
        with tc.tile_pool(name="gbuf", bufs=1, side="right") as g_pool, \
             tc.tile_pool(name="w1c", bufs=4, side="right") as w1pool, \
             tc.tile_pool(name="w2c", bufs=3, side="right") as w2pool, \
             tc.tile_pool(name="outp", bufs=4, side="right") as opool, \
             tc.tile_pool(name="psG", bufs=2, space="PSUM") as psG:
            g = [g_pool.tile([P, OWN], bf16, tag=f"g{m}", name=f"g{m}")
                 for m in range(NFF)]
            w1r = w1_d.rearrange("(ko ki) f -> ki ko f", ki=P)
            for mc in range(NFF):
                w1c = w1pool.tile([P, ND, P], bf16, tag="w1c")
                nc.sync.dma_start(out=w1c, in_=w1r[:, :, P * mc:P * mc + P])
                ups = psG.tile([P, OWN], f32, tag="ups")
                for i in range(ND):
                    nc.tensor.matmul(ups, w1c[:, i, :], h2[i],
                                     start=(i == 0), stop=(i == ND - 1))
                nc.scalar.activation(g[mc], ups, Gelu,
                                     bias=bias["b1"][:, mc:mc + 1])
            w2r = w2_d.rearrange("(ko ki) f -> ki ko f", ki=P)
            for oc in range(ND):
                w2c = w2pool.tile([P, NFF, P], bf16, tag="w2c")
                nc.sync.dma_start(out=w2c, in_=w2r[:, :, P * oc:P * oc + P])
                wps = psG.tile([P, OWN], f32, tag="ups")
                for k in range(NFF):
                    nc.tensor.matmul(wps, w2c[:, k, :], g[k],
                                     start=(k == 0), stop=(k == NFF - 1))
                of = opool.tile([P, OWN], f32, tag="of")
                nc.scalar.activation(of, wps, Identity,
                                     bias=bias["b2"][:, oc:oc + 1])
                nc.vector.tensor_add(of, of, x2f[oc])
                nc.sync.dma_start(out=out_d[P * oc:P * oc + P, :], in_=of)

        pool_close(h2_cm, x2_cm, xoF_cm, wgtE_cm, hT_cm, wk_cm)

    return nc


def host_prep(inputs):
    """Build per-core input maps + gather metadata. Pure numpy."""
    x = np.asarray(inputs["x"], np.float32)
    ln1_w = np.asarray(inputs["ln1_w"], np.float32)
    ln1_b = np.asarray(inputs["ln1_b"], np.float32)
    ln2_w = np.asarray(inputs["ln2_w"], np.float32)
    ln2_b = np.asarray(inputs["ln2_b"], np.float32)

    def cat_heads(w):
        return np.ascontiguousarray(
            np.transpose(np.asarray(w, np.float32), (1, 0, 2)).reshape(D, D))

    wq_c, wk_c, wv_c = (cat_heads(inputs[k]) for k in ("Wq", "Wk", "Wv"))
    bq_f = np.asarray(inputs["bq"], np.float32).reshape(-1)
    bk_f = np.asarray(inputs["bk"], np.float32).reshape(-1)
    bv_f = np.asarray(inputs["bv"], np.float32).reshape(-1)
    Wp = np.asarray(inputs["Wp"], np.float32)
    bp = np.asarray(inputs["bp"], np.float32)
    W1 = np.asarray(inputs["W1"], np.float32)
    b1 = np.asarray(inputs["b1"], np.float32)
    W2 = np.asarray(inputs["W2"], np.float32)
    b2 = np.asarray(inputs["b2"], np.float32)

    sc = 1.0 / np.sqrt(HS)
    wq_eff = ((ln1_w[:, None] * wq_c) * sc).astype(BF16)
    bq_eff = ((ln1_b @ wq_c + bq_f) * sc).astype(np.float32)
    wk_eff = (ln1_w[:, None] * wk_c).astype(BF16)
    bk_eff = (ln1_b @ wk_c + bk_f).astype(np.float32)
    wv_eff = (ln1_w[:, None] * wv_c).astype(BF16)
    bv_eff = (ln1_b @ wv_c + bv_f).astype(np.float32)
    wp_eff = Wp.astype(BF16)
    w1_eff = (ln2_w[:, None] * W1).astype(BF16)
    b1_eff = (ln2_b @ W1 + b1).astype(np.float32)
    w2_eff = W2.astype(BF16)

    def chunked(v, n):
        return np.ascontiguousarray(v.reshape(n, P).T).astype(np.float32)

    shared = {
        "wq": wq_eff, "wk": wk_eff, "wv": wv_eff, "wp": wp_eff,
        "w1": w1_eff, "w2": w2_eff,
        "bq": chunked(bq_eff, ND), "bk": chunked(bk_eff, ND),
        "bv": chunked(bv_eff, ND), "bp": chunked(bp, ND),
        "b1": chunked(b1_eff, NFF), "b2": chunked(b2, ND),
    }

    in_maps, gathers = [], []
    for c in range(N_CORES):
        b, j = c // 4, c % 4
        qA, qB = QW * j, QW * (7 - j)
        xT = np.ascontiguousarray(x[b].T).astype(BF16)
        xo_f = np.ascontiguousarray(
            np.concatenate([x[b, qA:qA + QW].T, x[b, qB:qB + QW].T],
                           axis=1)).astype(np.float32)
        # masks: kci<8 -> qA causal mask duplicated for both heads;
        #        kci>=8 -> qB causal mask duplicated.
        ks = np.arange(P)[:, None]
        qs = np.arange(QW)[None, :]
        masks = np.zeros((NKC1, P, 2 * QW), np.float32)
        for kc in range(NKC1):
            if kc < NKC0:
                m = (P * kc + ks) <= (qA + qs)
            else:
                m = (P * kc + ks) <= (qB + qs)
            masks[kc, :, 0:QW] = m
            masks[kc, :, QW:2 * QW] = m
        m = dict(shared)
        m["xT"] = xT
        m["xo"] = xo_f.astype(BF16)
        m["xof"] = xo_f
        m["masks"] = masks.astype(BF16)
        in_maps.append(m)
        gathers.append((b, qA, qB))
    return in_maps, gathers


def make_nc():
    from concourse import bacc

    nc = bacc.Bacc("TRN2")
    build(nc)
    nc.compile()
    return nc


def kernel(**inputs):
    from concourse.bass_utils import run_bass_kernel_spmd

    nc = make_nc()
    in_maps, gathers = host_prep(inputs)
    res = run_bass_kernel_spmd(nc, in_maps, list(range(N_CORES)))
    out = np.zeros((B, S, D), np.float32)
    for c, (b, qA, qB) in enumerate(gathers):
        oT = res.results[c]["outT"]
        out[b, qA:qA + QW] = oT[:, 0:QW].T
        out[b, qB:qB + QW] = oT[:, QW:2 * QW].T
    return out
